# revision 1
# baseline (speedup 1.0000x reference)
"""TRN2 Bass kernel for nn_CMoE_25271587570017 (moe_routing).

Strategy: data-parallel over batch on 8 NeuronCores (B=1024 -> 128/core).
Per core:
  Gate at fp32-level precision (top-k selection is discontinuous, so logits
  must track the fp32 reference closely; a single swapped expert costs ~1-3%
  global error):
    conv3x3 as 9 tap-matmuls in 3-term compensated float32r
    (w_r*x_r + w_r*x_lo + w_lo*x_r, residual ~2^-26) on an (i,j,b)-ordered
    zero-bordered 8x8 canvas -> relu -> maxpool (2 tensor-max ops) ->
    fc1/fc2 in plain fp32 (contiguous moving operands) -> top-2 softmax
    routing weights w[b,e] (0 for unselected).
  float32r ISA rules honored throughout: moving operand innermost count even
  (batch innermost), psum dest innermost contiguous+even, outer strides even,
  start_partition 0. PSUM start=True clears the whole bank, so only the first
  matmul per bank sets it.
  Experts in float32r (TF32-ish precision, full PE rate at N>=256), computed
  densely for all 8 experts in pairs stacked along PE partitions; routing is
  applied as out = sum_e relu(w[b,e]*(conv2_e + t_e)) == sum_e w[b,e]*relu(...)
  since w >= 0, so no gather/scatter is needed:
    dconv (stride-2 transpose conv) via parity-grid decomposition: tap (ti,tj)
      writes parity grid (ti%2, tj%2) at offset (ti//2, tj//2); grids are 7x7
      per sample in PSUM, evicted with fused relu+bias into a zero-bordered
      14x14 y-canvas (f32r) that conv2 reads with pad=1 for free.
    conv2 3x3 as 9 tap-matmuls, expert pair block-diagonal ([y_A;y_B] along K,
      [outA;outB] along M), BN scale folded into weights, BN shift+conv bias
      folded into the eviction bias.
  Epilogue: r = relu(psum + t); acc += w_bcast * r (w broadcast to partitions
  via a tiny selector matmul); final out = acc[0:64] + acc[64:128].
"""
import numpy as np
from contextlib import ExitStack

import concourse.bass as bass
import concourse.bacc as bacc
import concourse.tile as tile
from concourse import mybir
from concourse.bass_utils import run_bass_kernel_spmd

F32 = mybir.dt.float32
F32R = mybir.dt.float32r
AF = mybir.ActivationFunctionType
OP = mybir.AluOpType

NCORES = 8
B, BS = 1024, 128          # full batch, per-core shard
CIN, CO, E, NPAIR = 128, 64, 8, 4
BN_EPS = 1e-5
BLK = 16                   # samples per expert block
NBLK = BS // BLK
SUB = 8                    # dconv sub-chunk (samples); N = 8*36 = 288 >= 256
C2CH = (3, 3, 3, 3, 2, 2)  # conv2 sub-chunks per block (N = 432/288)

_CACHE = {}


def _tap_order(parity_taps):
    # start=True tap must cover the evicted region: highest (ti,tj) first
    return sorted(parity_taps, key=lambda t: (-t[0], -t[1]))


def _build(top_k: int, debug: bool = False):
    nc = bacc.Bacc("TRN2", target_bir_lowering=False, debug=False)

    x_d = nc.declare_dram_parameter("x", [BS, CIN, 6, 6], F32, isOutput=False)
    gt_d = nc.declare_dram_parameter("g_taps", [9, 128, 128], F32, isOutput=False)
    gb_d = nc.declare_dram_parameter("g_bias", [128, 1], F32, isOutput=False)
    f1_d = nc.declare_dram_parameter("fc1_t", [9, 128, 256], F32, isOutput=False)
    f1b_d = nc.declare_dram_parameter("fc1_bias", [2, 128, 1], F32, isOutput=False)
    f2_d = nc.declare_dram_parameter("fc2_t", [2, 128, 8], F32, isOutput=False)
    f2b_d = nc.declare_dram_parameter("fc2_bias", [8, 1], F32, isOutput=False)
    wd_d = nc.declare_dram_parameter("wd_t", [NPAIR, 9, 128, 128], F32, isOutput=False)
    wc_d = nc.declare_dram_parameter("wc_t", [NPAIR, 9, 128, 128], F32, isOutput=False)
    sel_d = nc.declare_dram_parameter("sel_t", [NPAIR, 8, 128], F32, isOutput=False)
    bd_d = nc.declare_dram_parameter("bd_t", [NPAIR, 128, 1], F32, isOutput=False)
    tt_d = nc.declare_dram_parameter("tt_t", [NPAIR, 128, 1], F32, isOutput=False)
    out_d = nc.declare_dram_parameter("out", [BS, CO, 144], F32, isOutput=True)
    if debug:
        dbg_lg = nc.declare_dram_parameter("dbg_lg", [128, 8], F32, isOutput=True)
        dbg_w = nc.declare_dram_parameter("dbg_w", [128, 8], F32, isOutput=True)
        dbg_wb = nc.declare_dram_parameter("dbg_wb", [128, NPAIR * 128], F32, isOutput=True)
        dbg_h = nc.declare_dram_parameter("dbg_h", [128, 36 * BS], F32, isOutput=True)
        dbg_p = nc.declare_dram_parameter("dbg_p", [128, 9 * BS], F32, isOutput=True)
        dbg_z = nc.declare_dram_parameter("dbg_z", [128, 256], F32, isOutput=True)

    with tile.TileContext(nc) as tc, ExitStack() as ctx:
        const = ctx.enter_context(tc.tile_pool(name="const", bufs=1))
        work = ctx.enter_context(tc.tile_pool(name="work", bufs=1))
        rp = ctx.enter_context(tc.tile_pool(name="rp", bufs=2))
        tmpp = ctx.enter_context(tc.tile_pool(name="tmpp", bufs=1))
        ps = ctx.enter_context(tc.tile_pool(name="ps", bufs=2, space="PSUM"))

        # ---------------- constants ----------------
        f1_sb = const.tile([128, 9 * 256], F32)
        nc.sync.dma_start(f1_sb[:].rearrange("p (t c) -> p t c", t=9),
                          f1_d[:].transpose([1, 0, 2]))
        f2_sb = const.tile([128, 2 * 8], F32)
        nc.sync.dma_start(f2_sb[:].rearrange("p (t c) -> p t c", t=2),
                          f2_d[:].transpose([1, 0, 2]))
        gb_sb = const.tile([128, 1], F32)
        nc.sync.dma_start(gb_sb[:], gb_d[:])
        f1b_sb = const.tile([128, 2], F32)
        nc.sync.dma_start(f1b_sb[:].rearrange("p (t c) -> p t c", t=2),
                          f1b_d[:].transpose([1, 0, 2]))
        f2b_sb = const.tile([8, 1], F32)
        nc.sync.dma_start(f2b_sb[:], f2b_d[:])
        bd_sb = const.tile([128, NPAIR], F32)
        nc.sync.dma_start(bd_sb[:].rearrange("p (t c) -> p t c", t=NPAIR),
                          bd_d[:].transpose([1, 0, 2]))
        tt_sb = const.tile([128, NPAIR], F32)
        nc.sync.dma_start(tt_sb[:].rearrange("p (t c) -> p t c", t=NPAIR),
                          tt_d[:].transpose([1, 0, 2]))

        # x: contiguous stage, then (i,j,b)-ordered zero-bordered f32r canvases
        # xcr = round(x) on an 8x8 canvas, xclo = x - xcr (compensation term)
        xs = work.tile([128, BS * 36], F32, tag="xs")
        nc.sync.dma_start(xs[:].rearrange("p (b s) -> p b s", b=BS),
                          x_d[:].rearrange("b p i j -> b p (i j)").transpose([1, 0, 2]))
        wstage3 = work.tile([128, 9 * 128], F32, tag="wstage")
        nc.sync.dma_start(wstage3[:, 0:9 * 128].rearrange("p (t c) -> p t c", t=9),
                          gt_d[:].transpose([1, 0, 2]))
        gt_r = const.tile([128, 9 * 128], F32R)
        nc.vector.tensor_copy(gt_r[:], wstage3[:, 0:9 * 128])
        gt_lo = const.tile([128, 9 * 128], F32R)
        nc.vector.tensor_tensor(gt_lo[:], wstage3[:, 0:9 * 128], gt_r[:], op=OP.subtract)

        wstage = work.tile([128, NPAIR * 9 * 128], F32, tag="wstage")
        nc.sync.dma_start(wstage[:].rearrange("p (a t c) -> p a t c", a=NPAIR, t=9),
                          wd_d[:].transpose([2, 0, 1, 3]))
        wd_r = const.tile([128, NPAIR * 9 * 128], F32R)
        nc.vector.tensor_copy(wd_r[:], wstage[:])
        wstage2 = work.tile([128, NPAIR * 9 * 128], F32, tag="wstage")
        nc.sync.dma_start(wstage2[:].rearrange("p (a t c) -> p a t c", a=NPAIR, t=9),
                          wc_d[:].transpose([2, 0, 1, 3]))
        wc_r = const.tile([128, NPAIR * 9 * 128], F32R)
        nc.vector.tensor_copy(wc_r[:], wstage2[:])

        selstage = work.tile([8, NPAIR * 128], F32, tag="wstage")
        nc.sync.dma_start(selstage[:].rearrange("p (a c) -> p a c", a=NPAIR),
                          sel_d[:].transpose([1, 0, 2]))
        sel_r = const.tile([8, NPAIR * 128], F32R)
        nc.vector.tensor_copy(sel_r[:], selstage[:])

        from concourse.masks import make_identity
        ident = const.tile([128, 128], F32)
        make_identity(nc, ident[:])

        xsv = xs[:].rearrange("p (b i j) -> p b i j", b=BS, i=6, j=6)
        xs_t = xsv.transpose([0, 2, 3, 1])            # (p, u, v, b)
        xcr = work.tile([128, 64 * BS], F32R, tag="xcr")
        nc.gpsimd.memset(xcr[:].bitcast(F32), 0.0)
        xcrv = xcr[:].rearrange("p (i j b) -> p i j b", i=8, j=8)
        nc.vector.tensor_copy(xcrv[:, 1:7, 1:7, :], xs_t)
        xclo = work.tile([128, 64 * BS], F32R, tag="xclo")
        nc.gpsimd.memset(xclo[:].bitcast(F32), 0.0)
        xclov = xclo[:].rearrange("p (i j b) -> p i j b", i=8, j=8)
        nc.vector.tensor_tensor(xclov[:, 1:7, 1:7, :], xs_t, xcrv[:, 1:7, 1:7, :],
                                op=OP.subtract)

        # ---------------- gate ----------------
        # h layout: (u, v, b) with b innermost (f32r moving/dst rules)
        h_sb = work.tile([128, BS * 36], F32, tag="xs")
        hsv = h_sb[:].rearrange("p (i j b) -> p i j b", i=6, j=6)
        gchunks = []
        _b0 = 0
        for gsz in [14] * 9 + [2]:
            gchunks.append((_b0, gsz))
            _b0 += gsz
        for b0, GCH in gchunks:
            hps = ps.tile([128, 2048], F32, tag="ps")
            hview = hps[:, 0:GCH * 36].rearrange("p (i j b) -> p i j b", i=6, j=6)
            first = True
            for di in range(3):
                for dj in range(3):
                    t = di * 3 + dj
                    rhs_r = xcrv[:, di:di + 6, dj:dj + 6, b0:b0 + GCH]
                    rhs_lo = xclov[:, di:di + 6, dj:dj + 6, b0:b0 + GCH]
                    nc.tensor.matmul(hview[:], gt_r[:, t * 128:(t + 1) * 128],
                                     rhs_r, start=first, stop=False)
                    nc.tensor.matmul(hview[:], gt_r[:, t * 128:(t + 1) * 128],
                                     rhs_lo, start=False, stop=False)
                    nc.tensor.matmul(hview[:], gt_lo[:, t * 128:(t + 1) * 128],
                                     rhs_r, start=False, stop=(t == 8))
                    first = False
            nc.scalar.activation(hsv[:, :, :, b0:b0 + GCH],
                                 hps[:, 0:GCH * 36].rearrange("p (i j b) -> p i j b", i=6, j=6),
                                 AF.Relu, bias=gb_sb[:], scale=1.0)

        # ---------------- experts ----------------
        # tap -> parity grid bookkeeping
        par_taps = {}
        for ti in range(3):
            for tj in range(3):
                par_taps.setdefault((ti % 2, tj % 2), []).append((ti, tj))

        y_full = work.tile([128, 64 * BS], F32, tag="xclo")
        nc.gpsimd.memset(y_full[:, 0:2 * BLK * 196], 0.0)
        y_store = y_full[:, 0:2 * BLK * 196].bitcast(mybir.dt.float32r)

        def emit_dconv(blk, pr):
            slot = (blk * NPAIR + pr) % 2
            yslot = y_store[:, slot * BLK * 196:(slot + 1) * BLK * 196]
            yv = yslot.rearrange("p (b c d) -> p b c d", b=BLK, c=14, d=14)
            for sub in range(BLK // SUB):
                b0 = blk * BLK + sub * SUB
                cps = ps.tile([128, 2048], F32, tag="ps")
                for (s_, t_), taps in par_taps.items():
                    bank = cps[:, (s_ * 2 + t_) * 512:(s_ * 2 + t_ + 1) * 512]
                    gv = bank.rearrange("p (u v b) -> p u v b", u=8, v=8)
                    for k, (ti, tj) in enumerate(_tap_order(taps)):
                        oi, oj = ti // 2, tj // 2
                        nc.tensor.matmul(
                            gv[:, oi:oi + 6, oj:oj + 6, :],
                            wd_r[:, (pr * 9 + ti * 3 + tj) * 128:(pr * 9 + ti * 3 + tj + 1) * 128],
                            xcrv[:, 1:7, 1:7, b0:b0 + SUB],
                            start=(k == 0), stop=(k == len(taps) - 1))
                for (s_, t_) in par_taps:
                    bank = cps[:, (s_ * 2 + t_) * 512:(s_ * 2 + t_ + 1) * 512]
                    gv = bank.rearrange("p (u v b) -> p u v b", u=8, v=8)
                    src = gv[:, (1 - s_):(1 - s_) + 6, (1 - t_):(1 - t_) + 6, :]
                    src = src.transpose([0, 3, 1, 2])
                    dst = yv[:, sub * SUB:(sub + 1) * SUB,
                             (2 - s_):14 - s_:2, (2 - t_):14 - t_:2]
                    nc.scalar.activation(dst, src, AF.Relu,
                                         bias=bd_sb[:, pr:pr + 1], scale=1.0)

        # hoist first two dconv pair-groups to hide the gate FC/top-2 tail
        emit_dconv(0, 0)
        emit_dconv(0, 1)

        hm_full = work.tile([128, NPAIR * 9 * 128], F32, tag="wstage")
        hm = hm_full[:, 0:BS * 18]
        hmv = hm[:].rearrange("p (i j b) -> p i j b", i=6, j=3)
        nc.vector.tensor_tensor(hmv[:], hsv[:, :, 0:6:2, :], hsv[:, :, 1:6:2, :], op=OP.max)
        p_sb = work.tile([128, BS * 9], F32, tag="p_sb")
        pv = p_sb[:].rearrange("p (i j b) -> p i j b", i=3, j=3)
        nc.vector.tensor_tensor(pv[:], hmv[:, 0:6:2, :, :], hmv[:, 1:6:2, :, :], op=OP.max)

        zt = ps.tile([128, 2048], F32, tag="ps")
        for s in range(9):
            for hh in range(2):
                nc.tensor.matmul(zt[:, hh * 128:(hh + 1) * 128],
                                 f1_sb[:, s * 256 + hh * 128: s * 256 + (hh + 1) * 128],
                                 p_sb[:, s * 128:(s + 1) * 128],
                                 start=(s == 0 and hh == 0), stop=(s == 8))
        z_sb = work.tile([128, 256], F32, tag="z_sb")
        for hh in range(2):
            nc.scalar.activation(z_sb[:, hh * 128:(hh + 1) * 128],
                                 zt[:, hh * 128:(hh + 1) * 128],
                                 AF.Relu, bias=f1b_sb[:, hh:hh + 1], scale=1.0)

        lgt = ps.tile([128, 2048], F32, tag="ps")
        for hh in range(2):
            nc.tensor.matmul(lgt[0:8, 0:128], f2_sb[:, hh * 8:(hh + 1) * 8],
                             z_sb[:, hh * 128:(hh + 1) * 128],
                             start=(hh == 0), stop=(hh == 1))
        lg_sb = work.tile([8, 128], F32, tag="lg_sb")
        nc.scalar.activation(lg_sb[:], lgt[0:8, 0:128], AF.Identity,
                             bias=f2b_sb[:], scale=1.0)

        # transpose logits -> [b, e]
        tps = ps.tile([128, 2048], F32, tag="ps")
        nc.tensor.transpose(tps[:, 0:8], lg_sb[:], ident[0:8, 0:8])
        lgb = work.tile([128, 8], F32, tag="lgb")
        nc.scalar.copy(lgb[:], tps[:, 0:8])

        # top-2 softmax weights (w[b,e] = 0 unless top-2)
        m1 = work.tile([128, 1], F32, tag="m1")
        nc.vector.tensor_reduce(m1[:], lgb[:], axis=mybir.AxisListType.X, op=OP.max)
        w_sb = work.tile([128, 8], F32, tag="w_sb")
        if top_k == 1:
            eq1 = work.tile([128, 8], F32, tag="eq1")
            nc.vector.tensor_scalar(eq1[:], lgb[:], m1[:], None, op0=OP.is_ge)
            den = work.tile([128, 1], F32, tag="den")
            nc.vector.tensor_reduce(den[:], eq1[:], axis=mybir.AxisListType.X, op=OP.add)
            rden = work.tile([128, 1], F32, tag="rden")
            nc.vector.reciprocal(rden[:], den[:])
            nc.vector.tensor_scalar(w_sb[:], eq1[:], rden[:], None, op0=OP.mult)
        else:
            assert top_k == 2, f"only top_k in (1,2) supported, got {top_k}"
            eq1 = work.tile([128, 8], F32, tag="eq1")
            nc.vector.tensor_scalar(eq1[:], lgb[:], m1[:], None, op0=OP.is_ge)
            msk = work.tile([128, 8], F32, tag="msk")
            nc.vector.scalar_tensor_tensor(msk[:], eq1[:], -1e30, lgb[:],
                                           op0=OP.mult, op1=OP.add)
            m2 = work.tile([128, 1], F32, tag="m2")
            nc.vector.tensor_reduce(m2[:], msk[:], axis=mybir.AxisListType.X, op=OP.max)
            sel2 = work.tile([128, 8], F32, tag="sel2")
            nc.vector.tensor_scalar(sel2[:], lgb[:], m2[:], None, op0=OP.is_ge)
            nm1 = work.tile([128, 1], F32, tag="nm1")
            nc.vector.tensor_scalar(nm1[:], m1[:], -1.0, None, op0=OP.mult)
            ex = work.tile([128, 8], F32, tag="ex")
            nc.scalar.activation(ex[:], lgb[:], AF.Exp, bias=nm1[:], scale=1.0)
            wun = work.tile([128, 8], F32, tag="wun")
            nc.vector.tensor_tensor(wun[:], ex[:], sel2[:], op=OP.mult)
            den = work.tile([128, 1], F32, tag="den")
            nc.vector.tensor_reduce(den[:], wun[:], axis=mybir.AxisListType.X, op=OP.add)
            rden = work.tile([128, 1], F32, tag="rden")
            nc.vector.reciprocal(rden[:], den[:])
            nc.vector.tensor_scalar(w_sb[:], wun[:], rden[:], None, op0=OP.mult)

        # transpose w -> [e, b], cast f32r, broadcast to partitions per pair
        tps2 = ps.tile([128, 2048], F32, tag="ps")
        nc.tensor.transpose(tps2[0:8, 0:128], w_sb[:], ident[:, :])
        wT = work.tile([8, 128], F32, tag="wT")
        nc.scalar.copy(wT[:], tps2[0:8, 0:128])
        wT_r = work.tile([8, 128], F32R, tag="wT_r")
        nc.vector.tensor_copy(wT_r[:], wT[:])
        wb_sb = const.tile([128, NPAIR * 128], F32)
        wps = ps.tile([128, 2048], F32, tag="ps")
        for pr in range(NPAIR):
            nc.tensor.matmul(wps[:, pr * 512:pr * 512 + 128],
                             sel_r[:, pr * 128:(pr + 1) * 128],
                             wT_r[:], start=True, stop=True)
        nc.scalar.copy(wb_sb[:].rearrange("p (a c) -> p a c", a=NPAIR),
                       wps[:].rearrange("p (a c) -> p a c", a=NPAIR)[:, :, 0:128])

        if debug:
            nc.sync.dma_start(dbg_lg[:], lgb[:])
            nc.sync.dma_start(dbg_w[:], w_sb[:])
            nc.sync.dma_start(dbg_wb[:], wb_sb[:])
            nc.sync.dma_start(dbg_h[:], h_sb[:])
            nc.sync.dma_start(dbg_p[:], p_sb[:])
            nc.sync.dma_start(dbg_z[:], z_sb[:])

        for blk in range(NBLK):
            acc = tmpp.tile([128, BLK * 144], F32, tag="acc")
            accv = acc[:].rearrange("p (b i j) -> p b i j", b=BLK, i=12, j=12)
            for pr in range(NPAIR):
                slot = (blk * NPAIR + pr) % 2
                yslot = y_store[:, slot * BLK * 196:(slot + 1) * BLK * 196]
                yv = yslot.rearrange("p (b c d) -> p b c d", b=BLK, c=14, d=14)
                if not (blk == 0 and pr < 2):
                    emit_dconv(blk, pr)
                # --- conv2 + epilogue ---
                rblk = rp.tile([128, BLK * 144], F32, tag="r_sb")
                cb0 = 0
                for ci_, cn in enumerate(C2CH):
                    c2 = ps.tile([128, 2048], F32, tag="ps")
                    oview = c2[:, 0:cn * 144]
                    first = True
                    for di in range(3):
                        for dj in range(3):
                            rhs = yv[:, cb0:cb0 + cn, di:di + 12, dj:dj + 12]
                            nc.tensor.matmul(
                                oview[:],
                                wc_r[:, (pr * 9 + di * 3 + dj) * 128:(pr * 9 + di * 3 + dj + 1) * 128],
                                rhs, start=first, stop=(di == 2 and dj == 2))
                            first = False
                    if ci_ % 2 == 0:
                        nc.scalar.activation(rblk[:, cb0 * 144:(cb0 + cn) * 144], oview[:],
                                             AF.Relu, bias=tt_sb[:, pr:pr + 1], scale=1.0)
                    else:
                        nc.vector.tensor_scalar(rblk[:, cb0 * 144:(cb0 + cn) * 144],
                                                oview[:], tt_sb[:, pr:pr + 1], 0.0,
                                                op0=OP.add, op1=OP.max)
                    cb0 += cn
                # whole-block weighted accumulation
                wv = wb_sb[:, pr * 128 + blk * BLK: pr * 128 + blk * BLK + BLK]
                wvb = wv.unsqueeze(2).broadcast_to([128, BLK, 144])
                rv = rblk[:].rearrange("p (b s) -> p b s", b=BLK)
                av = acc[:].rearrange("p (b s) -> p b s", b=BLK)
                if pr == 0:
                    nc.vector.tensor_tensor(av, rv, wvb, op=OP.mult)
                else:
                    tmp = tmpp.tile([128, BLK * 144], F32, tag="tmp")
                    tv = tmp[:].rearrange("p (b s) -> p b s", b=BLK)
                    nc.vector.tensor_tensor(tv, rv, wvb, op=OP.mult)
                    nc.vector.tensor_tensor(av, av, tv, op=OP.add)
            # final: sum expert-pair halves, DMA out
            a_hi = work.tile([64, BLK * 144], F32, tag="p_sb")
            nc.scalar.copy(a_hi[:], acc[64:128, :])
            o_sb_full = work.tile([64, NPAIR * 9 * 128], F32, tag="wstage")
            o_sb = o_sb_full[:, 0:BLK * 144]
            nc.vector.tensor_tensor(o_sb, acc[0:64, :], a_hi[:], op=OP.add)
            nc.sync.dma_start(
                out_d[blk * BLK:(blk + 1) * BLK].transpose([1, 0, 2]),
                o_sb.rearrange("p (b s) -> p b s", b=BLK))

    nc.finalize()
    return nc


def _prep(inputs):
    gw = np.asarray(inputs["gw"], np.float32)
    gb = np.asarray(inputs["gb"], np.float32)
    fc1_w = np.asarray(inputs["fc1_w"], np.float32)
    fc1_b = np.asarray(inputs["fc1_b"], np.float32)
    fc2_w = np.asarray(inputs["fc2_w"], np.float32)
    fc2_b = np.asarray(inputs["fc2_b"], np.float32)
    wd = np.asarray(inputs["wd"], np.float32)
    bd = np.asarray(inputs["bd"], np.float32)
    wc = np.asarray(inputs["wc"], np.float32)
    bc = np.asarray(inputs["bc"], np.float32)
    bn_g = np.asarray(inputs["bn_g"], np.float32)
    bn_b = np.asarray(inputs["bn_b"], np.float32)
    bn_m = np.asarray(inputs["bn_m"], np.float32)
    bn_v = np.asarray(inputs["bn_v"], np.float32)

    g_taps = np.ascontiguousarray(
        gw.transpose(2, 3, 1, 0).reshape(9, 128, 128))          # [t, ci, co]
    fc1_t = np.ascontiguousarray(
        fc1_w.reshape(256, 128, 9).transpose(2, 1, 0))           # [s, ci(co), m]
    fc2_t = np.ascontiguousarray(
        fc2_w.reshape(8, 2, 128).transpose(1, 2, 0))             # [h, j, e]

    sc = bn_g / np.sqrt(bn_v + BN_EPS)                           # [E, CO]
    tt = (bc - bn_m) * sc + bn_b                                 # [E, CO]

    wd_t = np.zeros((NPAIR, 9, 128, 128), np.float32)
    wc_t = np.zeros((NPAIR, 9, 128, 128), np.float32)
    sel_t = np.zeros((NPAIR, 8, 128), np.float32)
    bd_t = np.zeros((NPAIR, 128, 1), np.float32)
    tt_t = np.zeros((NPAIR, 128, 1), np.float32)
    for pr in range(NPAIR):
        for k in range(2):
            e = 2 * pr + k
            # dconv lhsT: [ci, k*64+co] = wd[e, ci, co, ti, tj]
            wd_t[pr, :, :, k * 64:(k + 1) * 64] = (
                wd[e].transpose(2, 3, 0, 1).reshape(9, 128, 64))
            # conv2 block-diag lhsT: [(k,ci), (k,co)] = wc[e, co, ci]*sc[e,co]
            wcp = wc[e].transpose(2, 3, 1, 0).reshape(9, 64, 64) * sc[e][None, None, :]
            wc_t[pr, :, k * 64:(k + 1) * 64, k * 64:(k + 1) * 64] = wcp
            sel_t[pr, e, k * 64:(k + 1) * 64] = 1.0
            bd_t[pr, k * 64:(k + 1) * 64, 0] = bd[e]
            tt_t[pr, k * 64:(k + 1) * 64, 0] = tt[e]

    return {
        "g_taps": g_taps, "g_bias": gb.reshape(128, 1),
        "fc1_t": fc1_t, "fc1_bias": fc1_b.reshape(2, 128, 1),
        "fc2_t": fc2_t, "fc2_bias": fc2_b.reshape(8, 1),
        "wd_t": wd_t, "wc_t": wc_t, "sel_t": sel_t,
        "bd_t": bd_t, "tt_t": tt_t,
    }


def kernel(**inputs) -> np.ndarray:
    x = np.ascontiguousarray(np.asarray(inputs["x"], np.float32))
    top_k = int(np.asarray(inputs["top_k"]))
    assert x.shape == (B, CIN, 6, 6)
    if top_k <= 0:
        return np.zeros((B, CO, 12, 12), np.float32)

    if top_k not in _CACHE:
        _CACHE[top_k] = _build(top_k)
    nc = _CACHE[top_k]

    weights = _prep(inputs)
    in_maps = []
    for c in range(NCORES):
        m = dict(weights)
        m["x"] = np.ascontiguousarray(x[c * BS:(c + 1) * BS])
        in_maps.append(m)

    res = run_bass_kernel_spmd(nc, in_maps, list(range(NCORES)))
    out = np.concatenate([res.results[c]["out"] for c in range(NCORES)], axis=0)
    return np.ascontiguousarray(out.reshape(B, CO, 12, 12).astype(np.float32))


if __name__ == "__main__":
    import os
    os.environ.setdefault("JAX_PLATFORMS", "")
    import reference as R
    inputs = R.setup_inputs()
    inp = {k: np.asarray(v) if hasattr(v, "shape") else v for k, v in inputs.items()}
    out = kernel(**inp)
    print("kernel output:", out.shape, out.dtype)

